# revision 27
# baseline (speedup 1.0000x reference)
"""Trainium2 Bass kernel for EncoderwithProjection (masked ROI pooling +
4-projector dense-dispatch MLP with BatchNorm + routed selection).

Strategy: data-parallel over batch across 8 NeuronCores (32 batch items
per core). BN statistics are aggregated with two small cross-core
AllReduces (one per projector pair), placed so they overlap the dense
matmul of later projectors. Heavy matmuls run in bf16 with fp32 PSUM
accumulation.

Host-side prep is layout only: transpose x to [b, hw, emb], permute mask
slots so each projector's slots are contiguous, retile weights for
wide-row DMA, cast to bf16. All arithmetic (pooling, area normalization,
matmuls, BN, ReLU, routing) happens on device.
"""

import numpy as np
from ml_dtypes import bfloat16

import concourse.bass as bass
import concourse.bacc as bacc
import concourse.tile as tile
import concourse.mybir as mybir
from concourse import bass_utils

BS, EMB, RES = 256, 768, 14
HW = RES * RES                      # 196
NM, HID, OUT, NPROJ = 16, 4096, 256, 4
BN_EPS = 1e-5
N_CORES = 8
BSL = BS // N_CORES                 # 32 batch items per core
T = BSL * NM                        # 512 tokens per core
EC = EMB // 128                     # 6 emb chunks
HJ = HID // 128                     # 32 hid tiles
NQT = 4                             # W1 streamed in quarters of hid
HJQ = HJ // NQT                     # 8 hid tiles per quarter
HW_LO, HW_HI = 128, HW - 128        # 128 + 68
NQ = BSL // 4                       # 8 quads of 4 batch items

F32 = mybir.dt.float32
BF16 = mybir.dt.bfloat16
AX = mybir.AxisListType
ALU = mybir.AluOpType
AF = mybir.ActivationFunctionType

LAST_RESULTS = None                 # BassKernelResults of the last run (for profiling)


def _build(counts):
    """Build the 8-core SPMD program. counts = tokens-per-projector (sorted slot order)."""
    offs = np.concatenate([[0], np.cumsum(counts)])  # slot-group offsets
    nc = bacc.Bacc("TRN2", target_bir_lowering=False, debug=False,
                   num_devices=N_CORES)

    xlo_d = nc.dram_tensor("xlo", [NQ, 128, 4 * EMB], BF16, kind="ExternalInput").ap()
    xhi_d = nc.dram_tensor("xhi", [NQ, HW_HI, 4 * EMB], BF16, kind="ExternalInput").ap()
    mT_d = nc.dram_tensor("mT", [HW, T], BF16, kind="ExternalInput").ap()
    mQ_d = nc.dram_tensor("mQ", [NQ, 128, HW], F32, kind="ExternalInput").ap()
    w1_d = nc.dram_tensor("w1", [NPROJ, NQT, 128, EC * 128 * HJQ], BF16,
                          kind="ExternalInput").ap()
    w2_d = nc.dram_tensor("w2", [NPROJ, 128, HJ * OUT], BF16,
                          kind="ExternalInput").ap()
    b2_d = nc.dram_tensor("b2", [1, NPROJ * OUT], BF16, kind="ExternalInput").ap()
    gb_d = nc.dram_tensor("gb", [128, 256], F32, kind="ExternalInput").ap()
    mq_d = nc.dram_tensor("mq", [128, 64], BF16, kind="ExternalInput").ap()
    out_d = nc.dram_tensor("out", [T, OUT], F32, kind="ExternalOutput").ap()

    with tile.TileContext(nc) as tc:
        with (
            tc.tile_pool(name="const", bufs=1) as cpool,
            tc.tile_pool(name="persist", bufs=1) as ppool,
        ):
            # ---- constants ----
            mq = cpool.tile([128, 64], BF16, tag="mq")
            nc.sync.dma_start(mq[:], mq_d[:])
            gb = cpool.tile([128, 256], F32, tag="gb")
            nc.sync.dma_start(gb[:], gb_d[:])
            mt_lo = cpool.tile([128, T], BF16, tag="mt_lo")
            nc.sync.dma_start(mt_lo[:], mT_d[0:HW_LO, :])
            mt_hi = cpool.tile([HW_HI, T], BF16, tag="mt_hi")
            nc.sync.dma_start(mt_hi[:], mT_d[HW_LO:HW, :])
            b2sb = cpool.tile([1, NPROJ * OUT], BF16, tag="b2sb")
            nc.sync.dma_start(b2sb[:], b2_d[:])
            ones_bf = cpool.tile([1, 128], BF16, tag="ones")
            nc.gpsimd.memset(ones_bf[:], 1.0)
            zrow = cpool.tile([1, 512], BF16, tag="zrow")
            nc.gpsimd.memset(zrow[:], 0.0)
            zcol = cpool.tile([1, 128], BF16, tag="zcol")
            nc.gpsimd.memset(zcol[:], 0.0)
            eps_sb = cpool.tile([128, 1], F32, tag="eps")
            nc.gpsimd.memset(eps_sb[:], BN_EPS)

            # PE warmup burst: full-array matmuls at t=0 so the HAM
            # clock-gate reaches full rate before the (skinny, M=16) pooling
            # matmuls, which never register enough activity to warm it
            warmsb = cpool.tile([128, 512], BF16, tag="warmsb")
            nc.gpsimd.memset(warmsb[:], 0.0)

            # persistent SBUF buffers
            pooledT = [ppool.tile([128, T], BF16, tag=f"pT{i}", name=f"pT{i}")
                       for i in range(EC)]
            hsel = {}
            for p in range(NPROJ):
                cp = int(counts[p])
                if cp == 0:
                    continue
                hsel[p] = [ppool.tile([128, BSL * cp], BF16,
                                      tag=f"hs{p}_{j}", name=f"hs{p}_{j}")
                           for j in range(HJ)]
            stats = [ppool.tile([128, HJ * 6], F32, tag=f"st{p}", name=f"st{p}")
                     for p in range(NPROJ)]
            # per projector pair: packed local sums, and final affine coeffs
            ss_sb = [ppool.tile([128, 128], F32, tag=f"ss{a}", name=f"ss{a}")
                     for a in range(2)]
            ssr = [ppool.tile([128, 128], F32, tag=f"ssr{a}", name=f"ssr{a}")
                   for a in range(2)]
            scale_a = [ppool.tile([128, 64], F32, tag=f"sa{a}", name=f"sa{a}")
                       for a in range(2)]
            bias_a = [ppool.tile([128, 64], F32, tag=f"ba{a}", name=f"ba{a}")
                      for a in range(2)]

            # =========== Phase 1: pooling ===========
            with (
                tc.tile_pool(name="poolw", bufs=2) as wpool,
                tc.tile_pool(name="poolps", bufs=1, space="PSUM") as qps,
            ):
                wtile = qps.tile([128, 512], F32, tag="ptsm", name="warmtile", bufs=2)
                for _ in range(16):
                    nc.tensor.matmul(wtile[:], warmsb[:, 0:128], warmsb[:],
                                     start=True, stop=True,
                                     skip_group_check=True)
                for q in range(NQ):
                    mn_sb = wpool.tile([128, HW], F32, tag="mnq")
                    nc.sync.dma_start(mn_sb[:], mQ_d[q])
                    area = wpool.tile([128, 1], F32, tag="area")
                    nc.vector.reduce_sum(area[:], mn_sb[:], axis=AX.X)
                    nc.vector.tensor_scalar_max(area[:], area[:], 1.0)
                    inv = wpool.tile([128, 1], F32, tag="inv")
                    nc.vector.reciprocal(inv[:], area[:])
                    diag = wpool.tile([128, 64], BF16, tag="diag")
                    nc.vector.tensor_scalar_mul(diag[:], mq[:], inv[:])

                    xq_lo = wpool.tile([128, 4 * EMB], BF16, tag="xq_lo", bufs=3)
                    nc.sync.dma_start(xq_lo[:, 0:2 * EMB], xlo_d[q][:, 0:2 * EMB])
                    nc.sync.dma_start(xq_lo[:, 2 * EMB:], xlo_d[q][:, 2 * EMB:])
                    xq_hi = wpool.tile([HW_HI, 4 * EMB], BF16, tag="xq_hi", bufs=3)
                    nc.scalar.dma_start(xq_hi[:, 0:2 * EMB], xhi_d[q][:, 0:2 * EMB])
                    nc.scalar.dma_start(xq_hi[:, 2 * EMB:], xhi_d[q][:, 2 * EMB:])

                    pool_ps = qps.tile([128, EMB], F32, tag="poolq", bufs=3)
                    # zero the whole tile (incl. padding rows) via K=1 matmuls
                    nc.tensor.matmul(pool_ps[:, 0:512], zcol[:], zrow[:, 0:512],
                                     start=True, stop=False, skip_group_check=True)
                    nc.tensor.matmul(pool_ps[:, 512:EMB], zcol[:], zrow[:, 0:EMB - 512],
                                     start=True, stop=False, skip_group_check=True)
                    for k in range(4):
                        b = 4 * q + k
                        o = pool_ps[32 * k:32 * k + NM, :]
                        tp = (0, 32 * k)
                        mlo = mt_lo[:, NM * b:NM * (b + 1)]
                        mhi = mt_hi[:, NM * b:NM * (b + 1)]
                        xl = xq_lo[:, EMB * k:EMB * (k + 1)]
                        xh = xq_hi[:, EMB * k:EMB * (k + 1)]
                        nc.tensor.matmul(o[:, 0:512], mlo, xl[:, 0:512],
                                         start=False, stop=False, tile_position=tp,
                                         skip_group_check=True)
                        nc.tensor.matmul(o[:, 512:EMB], mlo, xl[:, 512:EMB],
                                         start=False, stop=False, tile_position=tp,
                                         skip_group_check=True)
                        nc.tensor.matmul(o[:, 0:512], mhi, xh[:, 0:512],
                                         start=False, stop=False, tile_position=tp,
                                         skip_group_check=True)
                        nc.tensor.matmul(o[:, 512:EMB], mhi, xh[:, 512:EMB],
                                         start=False, stop=True, tile_position=tp,
                                         skip_group_check=True)
                    for _ in range(4):
                        nc.tensor.matmul(wtile[:], warmsb[:, 0:128], warmsb[:],
                                         start=True, stop=True,
                                         skip_group_check=True)
                    pq = wpool.tile([128, EMB], BF16, tag="pq")
                    nc.scalar.copy(pq[:], pool_ps[:])
                    for i in range(EC):
                        pts = qps.tile([128, 64], F32, tag="ptsm", bufs=2,
                                       name="ptsm")
                        nc.tensor.matmul(pts[:],
                                         pq[:, 128 * i:128 * (i + 1)], diag[:],
                                         start=True, stop=True)
                        nc.vector.tensor_copy(pooledT[i][:, 64 * q:64 * q + 64],
                                              pts[:])

            # =========== Phase 2: dense W1 matmul + BN stats ===========
            with (
                tc.tile_pool(name="mainw", bufs=1) as mpool,
                tc.tile_pool(name="w1pool", bufs=4) as w1pool,
                tc.tile_pool(name="w2pool", bufs=2) as w2pool,
                tc.tile_pool(name="mainps", bufs=1, space="PSUM") as mps,
                tc.tile_pool(name="dram", bufs=1, space="DRAM") as dpool,
            ):
                ss_in = [dpool.tile([128, 128], F32, tag=f"ssin{a}", name=f"ssin{a}")
                         for a in range(2)]
                ss_out = [dpool.tile([128, 128], F32, tag=f"ssout{a}", name=f"ssout{a}")
                          for a in range(2)]
                w2t = {}
                for p in range(NPROJ):
                    cp = int(counts[p])
                    for qt in range(NQT):
                        w1q = w1pool.tile([128, EC * 128 * HJQ], BF16, tag="w1q",
                                          bufs=2)
                        nc.scalar.dma_start(w1q[:], w1_d[p, qt])
                        for jj in range(HJQ):
                            j = HJQ * qt + jj
                            hps = mps.tile([128, T], F32, tag="hps", bufs=4)
                            for i in range(EC):
                                nc.tensor.matmul(
                                    hps[:],
                                    w1q[:, (i * HJQ + jj) * 128:(i * HJQ + jj) * 128 + 128],
                                    pooledT[i][:],
                                    start=(i == 0), stop=(i == EC - 1))
                            nc.vector.bn_stats(stats[p][:, 6 * j:6 * (j + 1)], hps[:])
                            if cp > 0:
                                src = hps[:].rearrange("p (b n) -> p b n", n=NM)
                                src = src[:, :, int(offs[p]):int(offs[p]) + cp]
                                dst = hsel[p][j][:].rearrange("p (b k) -> p b k", k=cp)
                                nc.vector.tensor_copy(dst, src)
                    if p < 2 and cp > 0:   # prefetch W2 for the first pair
                        w2t[p] = w2pool.tile([128, HJ * OUT], BF16, tag="w2sb",
                                             name=f"w2sb{p}")
                        nc.scalar.dma_start(w2t[p][:], w2_d[p])
                    # convert bn_stats (count, mean, count*var) x2 -> S1', S2''
                    a, half = p // 2, (p % 2) * 64
                    st = stats[p][:].rearrange("p (j s) -> p j s", s=6)
                    m_e, m_o = st[:, :, 1], st[:, :, 4]
                    v_e, v_o = st[:, :, 2], st[:, :, 5]
                    ss = ss_sb[a]
                    tmp1 = mpool.tile([128, HJ], F32, tag="tmp1")
                    tmp2 = mpool.tile([128, HJ], F32, tag="tmp2")
                    nc.vector.tensor_add(ss[:, half:half + HJ], m_e, m_o)
                    nc.vector.tensor_mul(tmp1[:], m_e, m_e)
                    nc.vector.tensor_mul(tmp2[:], m_o, m_o)
                    nc.vector.tensor_add(tmp1[:], tmp1[:], tmp2[:])
                    nc.vector.tensor_scalar_mul(tmp1[:], tmp1[:], float(T // 2))
                    nc.vector.tensor_add(tmp2[:], v_e, v_o)
                    nc.vector.tensor_add(ss[:, half + HJ:half + 2 * HJ],
                                         tmp1[:], tmp2[:])
                    if p % 2 == 1:
                        # cross-core reduction of the packed pair stats (gpsimd
                        # queue so the sync HWDGE queues never head-block on CC)
                        nc.gpsimd.dma_start(ss_in[a][:], ss_sb[a][:])
                        nc.gpsimd.collective_compute(
                            "AllReduce", ALU.add,
                            replica_groups=[list(range(N_CORES))],
                            ins=[ss_in[a].opt()], outs=[ss_out[a].opt()])
                        nc.gpsimd.dma_start(ssr[a][:], ss_out[a][:])

                # =========== Phase 3: normalize + ReLU + W2 ===========
                # post-CC stats math, emitted after all bn_stats/converts so the
                # in-order DVE/ACT streams never block the j-loop above
                for a in range(2):
                    sview = ssr[a][:].rearrange("p (g d s) -> p g d s", g=2, d=2)
                    mu = mpool.tile([128, 64], F32, tag=f"mu{a}", name=f"mu{a}")
                    muv = mu[:].rearrange("p (g s) -> p g s", g=2)
                    nc.vector.tensor_scalar_mul(muv, sview[:, :, 0, :], 1.0 / 16.0)
                    e2 = mpool.tile([128, 64], F32, tag="e2")
                    nc.vector.tensor_scalar_mul(
                        e2[:].rearrange("p (g s) -> p g s", g=2),
                        sview[:, :, 1, :], 1.0 / float(N_CORES * T))
                    var = mpool.tile([128, 64], F32, tag="var")
                    nc.vector.tensor_mul(var[:], mu[:], mu[:])
                    nc.vector.tensor_sub(var[:], e2[:], var[:])
                    std = mpool.tile([128, 64], F32, tag="std")
                    nc.scalar.activation(std[:], var[:], AF.Sqrt, bias=eps_sb[:])
                    istd = mpool.tile([128, 64], F32, tag="istd")
                    nc.vector.reciprocal(istd[:], std[:])
                    nc.vector.tensor_mul(scale_a[a][:],
                                         gb[:, 64 * a:64 * (a + 1)], istd[:])
                    nc.vector.tensor_mul(mu[:], mu[:], scale_a[a][:])
                    nc.vector.tensor_sub(bias_a[a][:],
                                         gb[:, 128 + 64 * a:128 + 64 * (a + 1)],
                                         mu[:])
                for p in range(NPROJ):
                    cp = int(counts[p])
                    if cp == 0:
                        continue
                    a, half = p // 2, (p % 2) * 32
                    if p not in w2t:
                        w2t[p] = w2pool.tile([128, HJ * OUT], BF16, tag="w2sb",
                                             name=f"w2sb{p}")
                        nc.scalar.dma_start(w2t[p][:], w2_d[p])
                    for j in range(HJ):
                        # normalize + relu, split across ACT and DVE to shorten
                        # the post-collective tail
                        if j % 3 == 0:
                            nc.scalar.activation(
                                hsel[p][j][:], hsel[p][j][:], AF.Relu,
                                bias=bias_a[a][:, half + j:half + j + 1],
                                scale=scale_a[a][:, half + j:half + j + 1])
                        else:
                            nc.vector.tensor_scalar(
                                hsel[p][j][:], hsel[p][j][:],
                                scale_a[a][:, half + j:half + j + 1],
                                bias_a[a][:, half + j:half + j + 1],
                                op0=ALU.mult, op1=ALU.add)
                            nc.vector.tensor_scalar_max(hsel[p][j][:],
                                                        hsel[p][j][:], 0.0)
                    width = BSL * cp
                    for m0 in range(0, width, 128):
                        m = min(128, width - m0)
                        ops = mps.tile([128, OUT], F32, tag="ops", bufs=2)
                        for j in range(HJ):
                            nc.tensor.matmul(ops[0:m, :],
                                             hsel[p][j][:, m0:m0 + m],
                                             w2t[p][:, OUT * j:OUT * (j + 1)],
                                             start=(j == 0), stop=False,
                                             skip_group_check=True)
                        nc.tensor.matmul(ops[0:m, :], ones_bf[:, 0:m],
                                         b2sb[:, OUT * p:OUT * (p + 1)],
                                         start=False, stop=True,
                                         skip_group_check=True)
                        osb = mpool.tile([128, OUT], F32, tag="osb")
                        nc.scalar.copy(osb[0:m, :], ops[0:m, :])
                        r0 = 32 * int(offs[p]) + m0
                        nc.sync.dma_start(out_d[r0:r0 + m, :], osb[0:m, :])

    nc.compile()
    return nc


_CACHE = {}


def kernel(x, masks, projection_idx, mask_ids, W1, b1, gamma, beta, W2, b2):
    global LAST_RESULTS
    x = np.asarray(x, dtype=np.float32)
    masks = np.asarray(masks, dtype=np.float32)
    projection_idx = np.asarray(projection_idx, dtype=np.int32)
    W1 = np.asarray(W1, dtype=np.float32)
    gamma = np.asarray(gamma, dtype=np.float32)
    beta = np.asarray(beta, dtype=np.float32)
    W2 = np.asarray(W2, dtype=np.float32)
    b2 = np.asarray(b2, dtype=np.float32)

    # ---- host-side routing / layout prep ----
    perm = np.argsort(projection_idx, kind="stable")
    counts = np.bincount(projection_idx, minlength=NPROJ)
    offs = np.concatenate([[0], np.cumsum(counts)])

    masks_p = masks[:, perm, :]                                   # [BS, NM, HW]
    xT = x.reshape(BS, EMB, HW).transpose(0, 2, 1).astype(bfloat16)  # [BS, HW, EMB]
    # quad-packed x: [quad, hw, (b-in-quad, emb)]
    xq = xT.reshape(BS // 4, 4, HW, EMB).transpose(0, 2, 1, 3)
    xlo = np.ascontiguousarray(xq[:, :HW_LO]).reshape(BS // 4, HW_LO, 4 * EMB)
    xhi = np.ascontiguousarray(xq[:, HW_LO:]).reshape(BS // 4, HW_HI, 4 * EMB)
    # W1 retiled: [p, hid-quarter, emb%128, (emb-chunk, hid-tile-in-qt, 128)]
    w1b = np.ascontiguousarray(
        W1.reshape(NPROJ, EC, 128, NQT, HJQ * 128).transpose(0, 3, 2, 1, 4)
    ).reshape(NPROJ, NQT, 128, EC * HJQ * 128).astype(bfloat16)
    # W2 retiled: [p, hid%128, (hid-tile, out)]
    w2b = np.ascontiguousarray(
        W2.reshape(NPROJ, HJ, 128, OUT).transpose(0, 2, 1, 3)
    ).reshape(NPROJ, 128, HJ * OUT).astype(bfloat16)
    b2b = b2.reshape(1, NPROJ * OUT).astype(bfloat16)
    # gamma/beta packed as [128, p*32+j] (partition = hid % 128, col = (p, j))
    gr = np.ascontiguousarray(gamma.reshape(NPROJ, HJ, 128).transpose(2, 0, 1)
                              ).reshape(128, NPROJ * HJ)
    br = np.ascontiguousarray(beta.reshape(NPROJ, HJ, 128).transpose(2, 0, 1)
                              ).reshape(128, NPROJ * HJ)
    gb = np.concatenate([gr, br], axis=1).astype(np.float32)      # [128, 256]
    mq = np.zeros((128, 64), dtype=bfloat16)
    for k in range(4):
        for j in range(NM):
            mq[32 * k + j, 16 * k + j] = 1.0

    key = tuple(int(c) for c in counts)
    if key not in _CACHE:
        _CACHE[key] = _build(counts)
    nc = _CACHE[key]

    in_maps = []
    for c in range(N_CORES):
        sl = slice(BSL * c, BSL * (c + 1))
        qsl = slice(NQ * c, NQ * (c + 1))
        mp = masks_p[sl]                                          # [BSL, NM, HW]
        mT = np.ascontiguousarray(
            mp.transpose(2, 0, 1).reshape(HW, T)).astype(bfloat16)
        mQ = np.zeros((NQ, 128, HW), dtype=np.float32)
        for k in range(4):
            mQ[:, 32 * k:32 * k + NM, :] = mp.reshape(NQ, 4, NM, HW)[:, k]
        in_maps.append({
            "xlo": xlo[qsl], "xhi": xhi[qsl], "mT": mT, "mQ": mQ,
            "w1": w1b, "w2": w2b, "b2": b2b, "gb": gb, "mq": mq,
        })

    res = bass_utils.run_bass_kernel_spmd(nc, in_maps, core_ids=list(range(N_CORES)))
    LAST_RESULTS = res

    xs = np.empty((BS, NM, OUT), dtype=np.float32)
    for c in range(N_CORES):
        shard = res.results[c]["out"]                             # [T, OUT]
        for p in range(NPROJ):
            cp = int(counts[p])
            if cp == 0:
                continue
            blk = shard[32 * offs[p]:32 * offs[p] + 32 * cp].reshape(BSL, cp, OUT)
            xs[BSL * c:BSL * (c + 1), perm[offs[p]:offs[p] + cp], :] = blk
    return xs, np.asarray(mask_ids)


# revision 30
# speedup vs baseline: 1.0849x; 1.0849x over previous
"""Trainium2 Bass kernel for EncoderwithProjection (masked ROI pooling +
4-projector dense-dispatch MLP with BatchNorm + routed selection).

Strategy: data-parallel over batch across 8 NeuronCores (32 batch items
per core). BN statistics are aggregated with two small cross-core
AllReduces (one per projector pair), placed so they overlap the dense
matmul of later projectors. Heavy matmuls run in bf16 with fp32 PSUM
accumulation.

Host-side prep is layout only: transpose x to [b, hw, emb], permute mask
slots so each projector's slots are contiguous, retile weights for
wide-row DMA, cast to bf16. All arithmetic (pooling, area normalization,
matmuls, BN, ReLU, routing) happens on device.
"""

import numpy as np
from ml_dtypes import bfloat16

import concourse.bass as bass
import concourse.bacc as bacc
import concourse.tile as tile
import concourse.mybir as mybir
from concourse import bass_utils

BS, EMB, RES = 256, 768, 14
HW = RES * RES                      # 196
NM, HID, OUT, NPROJ = 16, 4096, 256, 4
BN_EPS = 1e-5
N_CORES = 8
BSL = BS // N_CORES                 # 32 batch items per core
T = BSL * NM                        # 512 tokens per core
EC = EMB // 128                     # 6 emb chunks
HJ = HID // 128                     # 32 hid tiles
NQT = 4                             # W1 streamed in quarters of hid
HJQ = HJ // NQT                     # 8 hid tiles per quarter
HW_LO, HW_HI = 128, HW - 128        # 128 + 68
NQ = BSL // 4                       # 8 quads of 4 batch items

F32 = mybir.dt.float32
BF16 = mybir.dt.bfloat16
AX = mybir.AxisListType
ALU = mybir.AluOpType
AF = mybir.ActivationFunctionType

LAST_RESULTS = None                 # BassKernelResults of the last run (for profiling)


def _build(counts):
    """Build the 8-core SPMD program. counts = tokens-per-projector (sorted slot order)."""
    offs = np.concatenate([[0], np.cumsum(counts)])  # slot-group offsets
    nc = bacc.Bacc("TRN2", target_bir_lowering=False, debug=False,
                   num_devices=N_CORES)

    xlo_d = nc.dram_tensor("xlo", [NQ, 128, 4 * EMB], BF16, kind="ExternalInput").ap()
    xhi_d = nc.dram_tensor("xhi", [NQ, HW_HI, 4 * EMB], BF16, kind="ExternalInput").ap()
    mQ_d = nc.dram_tensor("mQ", [NQ, 128, HW], F32, kind="ExternalInput").ap()
    w1_d = nc.dram_tensor("w1", [NPROJ, NQT, 128, EC * 128 * HJQ], BF16,
                          kind="ExternalInput").ap()
    w2_d = nc.dram_tensor("w2", [NPROJ, 128, HJ * OUT], BF16,
                          kind="ExternalInput").ap()
    b2_d = nc.dram_tensor("b2", [1, NPROJ * OUT], BF16, kind="ExternalInput").ap()
    gb_d = nc.dram_tensor("gb", [128, 256], F32, kind="ExternalInput").ap()
    i128_d = nc.dram_tensor("i128", [128, 128], BF16, kind="ExternalInput").ap()
    out_d = nc.dram_tensor("out", [T, OUT], F32, kind="ExternalOutput").ap()

    with tile.TileContext(nc) as tc:
        with (
            tc.tile_pool(name="const", bufs=1) as cpool,
            tc.tile_pool(name="persist", bufs=1) as ppool,
        ):
            # ---- constants ----
            i128 = cpool.tile([128, 128], BF16, tag="i128")
            nc.sync.dma_start(i128[:], i128_d[:])
            gb = cpool.tile([128, 256], F32, tag="gb")
            nc.sync.dma_start(gb[:], gb_d[:])
            b2sb = cpool.tile([1, NPROJ * OUT], BF16, tag="b2sb")
            nc.sync.dma_start(b2sb[:], b2_d[:])
            ones_bf = cpool.tile([1, 128], BF16, tag="ones")
            nc.gpsimd.memset(ones_bf[:], 1.0)
            eps_sb = cpool.tile([128, 1], F32, tag="eps")
            nc.gpsimd.memset(eps_sb[:], BN_EPS)

            # persistent SBUF buffers
            pooledT = ppool.tile([128, EC * T], BF16, tag="pooledT")
            hsel = {}
            for p in range(NPROJ):
                cp = int(counts[p])
                if cp == 0:
                    continue
                hsel[p] = [ppool.tile([128, BSL * cp], BF16,
                                      tag=f"hs{p}_{j}", name=f"hs{p}_{j}")
                           for j in range(HJ)]
            stats = [ppool.tile([128, HJ * 6], F32, tag=f"st{p}", name=f"st{p}")
                     for p in range(NPROJ)]
            # per projector pair: packed local sums, and final affine coeffs
            ss_sb = [ppool.tile([128, 128], F32, tag=f"ss{a}", name=f"ss{a}")
                     for a in range(2)]
            ssr = [ppool.tile([128, 128], F32, tag=f"ssr{a}", name=f"ssr{a}")
                   for a in range(2)]
            scale_a = [ppool.tile([128, 64], F32, tag=f"sa{a}", name=f"sa{a}")
                       for a in range(2)]
            bias_a = [ppool.tile([128, 64], F32, tag=f"ba{a}", name=f"ba{a}")
                      for a in range(2)]

            # =========== Phase 1: pooling ===========
            # smplT[hw, t] = masksT scaled by 1/area (via block-diagonal
            # matmuls against diag(inv_area)), then pooledT[e, t] directly via
            # M=128-wide, N=16 matmuls (LDWEIGHTS-rate bound, clock-gate
            # independent)
            with (
                tc.tile_pool(name="poolw", bufs=2) as wpool,
                tc.tile_pool(name="poolps", bufs=1, space="PSUM") as qps,
            ):
                smpl_lo = wpool.tile([128, NQ * 128], BF16, tag="smpl_lo", bufs=1)
                smpl_hi = wpool.tile([HW_HI, NQ * 128], BF16, tag="smpl_hi", bufs=1)
                for q in range(NQ):
                    mn_sb = wpool.tile([128, HW], F32, tag="mnq")
                    nc.sync.dma_start(mn_sb[:], mQ_d[q])
                    area = wpool.tile([128, 1], F32, tag="area")
                    nc.vector.reduce_sum(area[:], mn_sb[:], axis=AX.X)
                    nc.vector.tensor_scalar_max(area[:], area[:], 1.0)
                    inv = wpool.tile([128, 1], F32, tag="inv")
                    nc.vector.reciprocal(inv[:], area[:])
                    diagq = wpool.tile([128, 128], BF16, tag="diagq")
                    nc.vector.tensor_scalar_mul(diagq[:], i128[:], inv[:])
                    mn_bf = wpool.tile([128, HW], BF16, tag="mnbf")
                    nc.scalar.copy(mn_bf[:], mn_sb[:])
                    sps_lo = qps.tile([128, 128], F32, tag="sps_lo", bufs=2)
                    nc.tensor.matmul(sps_lo[:], mn_bf[:, 0:HW_LO], diagq[:],
                                     start=True, stop=True, skip_group_check=True)
                    nc.vector.tensor_copy(smpl_lo[:, 128 * q:128 * (q + 1)],
                                          sps_lo[:])
                    sps_hi = qps.tile([HW_HI, 128], F32, tag="sps_hi", bufs=2)
                    nc.tensor.matmul(sps_hi[:], mn_bf[:, HW_LO:HW], diagq[:],
                                     start=True, stop=True, skip_group_check=True)
                    nc.vector.tensor_copy(smpl_hi[:, 128 * q:128 * (q + 1)],
                                          sps_hi[:])
                for q in range(NQ):
                    xq_lo = wpool.tile([128, 4 * EMB], BF16, tag="xq_lo", bufs=3)
                    nc.sync.dma_start(xq_lo[:, 0:2 * EMB], xlo_d[q][:, 0:2 * EMB])
                    nc.sync.dma_start(xq_lo[:, 2 * EMB:], xlo_d[q][:, 2 * EMB:])
                    xq_hi = wpool.tile([HW_HI, 4 * EMB], BF16, tag="xq_hi", bufs=3)
                    nc.scalar.dma_start(xq_hi[:, 0:2 * EMB], xhi_d[q][:, 0:2 * EMB])
                    nc.scalar.dma_start(xq_hi[:, 2 * EMB:], xhi_d[q][:, 2 * EMB:])
                    for k in range(4):
                        b = 4 * q + k
                        c0 = 128 * q + 32 * k       # quad-padded column base
                        pb = qps.tile([128, EC * NM], F32, tag="pb", bufs=4)
                        for i in range(EC):
                            nc.tensor.matmul(
                                pb[:, NM * i:NM * (i + 1)],
                                xq_lo[:, EMB * k + 128 * i:EMB * k + 128 * (i + 1)],
                                smpl_lo[:, c0:c0 + NM],
                                start=True, stop=False, skip_group_check=True)
                            nc.tensor.matmul(
                                pb[:, NM * i:NM * (i + 1)],
                                xq_hi[:, EMB * k + 128 * i:EMB * k + 128 * (i + 1)],
                                smpl_hi[:, c0:c0 + NM],
                                start=False, stop=True, skip_group_check=True)
                        dst = pooledT[:].rearrange("p (i t) -> p i t", t=T)
                        dst = dst[:, :, NM * b:NM * (b + 1)]
                        nc.vector.tensor_copy(dst, pb[:].rearrange(
                            "p (i n) -> p i n", n=NM))

            # =========== Phase 2: dense W1 matmul + BN stats ===========
            with (
                tc.tile_pool(name="mainw", bufs=1) as mpool,
                tc.tile_pool(name="w1pool", bufs=3) as w1pool,
                tc.tile_pool(name="w2pool", bufs=2) as w2pool,
                tc.tile_pool(name="mainps", bufs=1, space="PSUM") as mps,
                tc.tile_pool(name="dram", bufs=1, space="DRAM") as dpool,
            ):
                ss_in = [dpool.tile([128, 128], F32, tag=f"ssin{a}", name=f"ssin{a}")
                         for a in range(2)]
                ss_out = [dpool.tile([128, 128], F32, tag=f"ssout{a}", name=f"ssout{a}")
                          for a in range(2)]
                w2t = {}
                pview = pooledT[:].rearrange("p (i t) -> p i t", t=T)
                for p in range(NPROJ):
                    cp = int(counts[p])
                    for qt in range(NQT):
                        w1q = w1pool.tile([128, EC * 128 * HJQ], BF16, tag="w1q",
                                          bufs=3)
                        nc.scalar.dma_start(w1q[:], w1_d[p, qt])
                        for jj in range(HJQ):
                            j = HJQ * qt + jj
                            hps = mps.tile([128, T], F32, tag="hps", bufs=4)
                            for i in range(EC):
                                nc.tensor.matmul(
                                    hps[:],
                                    w1q[:, (i * HJQ + jj) * 128:(i * HJQ + jj) * 128 + 128],
                                    pview[:, i, :],
                                    start=(i == 0), stop=(i == EC - 1))
                            nc.vector.bn_stats(stats[p][:, 6 * j:6 * (j + 1)], hps[:])
                            if cp > 0:
                                src = hps[:].rearrange("p (b n) -> p b n", n=NM)
                                src = src[:, :, int(offs[p]):int(offs[p]) + cp]
                                dst = hsel[p][j][:].rearrange("p (b k) -> p b k", k=cp)
                                nc.vector.tensor_copy(dst, src)
                    if p < 2 and cp > 0:   # prefetch W2 for the first pair
                        w2t[p] = w2pool.tile([128, HJ * OUT], BF16, tag="w2sb",
                                             name=f"w2sb{p}")
                        nc.scalar.dma_start(w2t[p][:], w2_d[p])
                    # convert bn_stats (count, mean, count*var) x2 -> S1', S2''
                    a, half = p // 2, (p % 2) * 64
                    st = stats[p][:].rearrange("p (j s) -> p j s", s=6)
                    m_e, m_o = st[:, :, 1], st[:, :, 4]
                    v_e, v_o = st[:, :, 2], st[:, :, 5]
                    ss = ss_sb[a]
                    tmp1 = mpool.tile([128, HJ], F32, tag="tmp1")
                    tmp2 = mpool.tile([128, HJ], F32, tag="tmp2")
                    nc.vector.tensor_add(ss[:, half:half + HJ], m_e, m_o)
                    nc.vector.tensor_mul(tmp1[:], m_e, m_e)
                    nc.vector.tensor_mul(tmp2[:], m_o, m_o)
                    nc.vector.tensor_add(tmp1[:], tmp1[:], tmp2[:])
                    nc.vector.tensor_scalar_mul(tmp1[:], tmp1[:], float(T // 2))
                    nc.vector.tensor_add(tmp2[:], v_e, v_o)
                    nc.vector.tensor_add(ss[:, half + HJ:half + 2 * HJ],
                                         tmp1[:], tmp2[:])
                    if p % 2 == 1:
                        # cross-core reduction of the packed pair stats (gpsimd
                        # queue so the HWDGE queues never head-block on CC)
                        nc.gpsimd.dma_start(ss_in[a][:], ss_sb[a][:])
                        nc.gpsimd.collective_compute(
                            "AllReduce", ALU.add,
                            replica_groups=[list(range(N_CORES))],
                            ins=[ss_in[a].opt()], outs=[ss_out[a].opt()])
                        nc.gpsimd.dma_start(ssr[a][:], ss_out[a][:])

                # =========== Phase 3: normalize + ReLU + W2 ===========
                # post-CC stats math, emitted after all bn_stats/converts so the
                # in-order DVE/ACT streams never block the j-loop above
                for a in range(2):
                    sview = ssr[a][:].rearrange("p (g d s) -> p g d s", g=2, d=2)
                    mu = mpool.tile([128, 64], F32, tag=f"mu{a}", name=f"mu{a}")
                    muv = mu[:].rearrange("p (g s) -> p g s", g=2)
                    nc.vector.tensor_scalar_mul(muv, sview[:, :, 0, :], 1.0 / 16.0)
                    e2 = mpool.tile([128, 64], F32, tag="e2")
                    nc.vector.tensor_scalar_mul(
                        e2[:].rearrange("p (g s) -> p g s", g=2),
                        sview[:, :, 1, :], 1.0 / float(N_CORES * T))
                    var = mpool.tile([128, 64], F32, tag="var")
                    nc.vector.tensor_mul(var[:], mu[:], mu[:])
                    nc.vector.tensor_sub(var[:], e2[:], var[:])
                    std = mpool.tile([128, 64], F32, tag="std")
                    nc.scalar.activation(std[:], var[:], AF.Sqrt, bias=eps_sb[:])
                    istd = mpool.tile([128, 64], F32, tag="istd")
                    nc.vector.reciprocal(istd[:], std[:])
                    nc.vector.tensor_mul(scale_a[a][:],
                                         gb[:, 64 * a:64 * (a + 1)], istd[:])
                    nc.vector.tensor_mul(mu[:], mu[:], scale_a[a][:])
                    nc.vector.tensor_sub(bias_a[a][:],
                                         gb[:, 128 + 64 * a:128 + 64 * (a + 1)],
                                         mu[:])

                for p in range(NPROJ):
                    cp = int(counts[p])
                    if cp == 0:
                        continue
                    a, half = p // 2, (p % 2) * 32
                    if p not in w2t:
                        w2t[p] = w2pool.tile([128, HJ * OUT], BF16, tag="w2sb",
                                             name=f"w2sb{p}")
                        nc.scalar.dma_start(w2t[p][:], w2_d[p])
                    for j in range(HJ):
                        # normalize + relu in place, split across ACT and DVE to
                        # shorten the post-collective tail
                        if j % 3 == 0:
                            nc.scalar.activation(
                                hsel[p][j][:], hsel[p][j][:], AF.Relu,
                                bias=bias_a[a][:, half + j:half + j + 1],
                                scale=scale_a[a][:, half + j:half + j + 1])
                        else:
                            nc.vector.tensor_scalar(
                                hsel[p][j][:], hsel[p][j][:],
                                scale_a[a][:, half + j:half + j + 1],
                                bias_a[a][:, half + j:half + j + 1],
                                op0=ALU.mult, op1=ALU.add)
                            nc.vector.tensor_scalar_max(hsel[p][j][:],
                                                        hsel[p][j][:], 0.0)
                    width = BSL * cp
                    for m0 in range(0, width, 128):
                        m = min(128, width - m0)
                        ops = mps.tile([128, OUT], F32, tag="ops", bufs=2)
                        for j in range(HJ):
                            nc.tensor.matmul(ops[0:m, :],
                                             hsel[p][j][:, m0:m0 + m],
                                             w2t[p][:, OUT * j:OUT * (j + 1)],
                                             start=(j == 0), stop=False,
                                             skip_group_check=True)
                        nc.tensor.matmul(ops[0:m, :], ones_bf[:, 0:m],
                                         b2sb[:, OUT * p:OUT * (p + 1)],
                                         start=False, stop=True,
                                         skip_group_check=True)
                        osb = mpool.tile([128, OUT], F32, tag="osb")
                        nc.scalar.copy(osb[0:m, :], ops[0:m, :])
                        r0 = 32 * int(offs[p]) + m0
                        nc.sync.dma_start(out_d[r0:r0 + m, :], osb[0:m, :])

    nc.compile()
    return nc


_CACHE = {}


def kernel(x, masks, projection_idx, mask_ids, W1, b1, gamma, beta, W2, b2):
    global LAST_RESULTS
    x = np.asarray(x, dtype=np.float32)
    masks = np.asarray(masks, dtype=np.float32)
    projection_idx = np.asarray(projection_idx, dtype=np.int32)
    W1 = np.asarray(W1, dtype=np.float32)
    gamma = np.asarray(gamma, dtype=np.float32)
    beta = np.asarray(beta, dtype=np.float32)
    W2 = np.asarray(W2, dtype=np.float32)
    b2 = np.asarray(b2, dtype=np.float32)

    # ---- host-side routing / layout prep ----
    perm = np.argsort(projection_idx, kind="stable")
    counts = np.bincount(projection_idx, minlength=NPROJ)
    offs = np.concatenate([[0], np.cumsum(counts)])

    masks_p = masks[:, perm, :]                                   # [BS, NM, HW]
    xT = x.reshape(BS, EMB, HW).transpose(0, 2, 1).astype(bfloat16)  # [BS, HW, EMB]
    # quad-packed x: [quad, hw, (b-in-quad, emb)]
    xq = xT.reshape(BS // 4, 4, HW, EMB).transpose(0, 2, 1, 3)
    xlo = np.ascontiguousarray(xq[:, :HW_LO]).reshape(BS // 4, HW_LO, 4 * EMB)
    xhi = np.ascontiguousarray(xq[:, HW_LO:]).reshape(BS // 4, HW_HI, 4 * EMB)
    # W1 retiled: [p, hid-quarter, emb%128, (emb-chunk, hid-tile-in-qt, 128)]
    w1b = np.ascontiguousarray(
        W1.reshape(NPROJ, EC, 128, NQT, HJQ * 128).transpose(0, 3, 2, 1, 4)
    ).reshape(NPROJ, NQT, 128, EC * HJQ * 128).astype(bfloat16)
    # W2 retiled: [p, hid%128, (hid-tile, out)]
    w2b = np.ascontiguousarray(
        W2.reshape(NPROJ, HJ, 128, OUT).transpose(0, 2, 1, 3)
    ).reshape(NPROJ, 128, HJ * OUT).astype(bfloat16)
    b2b = b2.reshape(1, NPROJ * OUT).astype(bfloat16)
    # gamma/beta packed as [128, p*32+j] (partition = hid % 128, col = (p, j))
    gr = np.ascontiguousarray(gamma.reshape(NPROJ, HJ, 128).transpose(2, 0, 1)
                              ).reshape(128, NPROJ * HJ)
    br = np.ascontiguousarray(beta.reshape(NPROJ, HJ, 128).transpose(2, 0, 1)
                              ).reshape(128, NPROJ * HJ)
    gb = np.concatenate([gr, br], axis=1).astype(np.float32)      # [128, 256]
    i128 = np.eye(128, dtype=bfloat16)

    key = tuple(int(c) for c in counts)
    if key not in _CACHE:
        _CACHE[key] = _build(counts)
    nc = _CACHE[key]

    in_maps = []
    for c in range(N_CORES):
        qsl = slice(NQ * c, NQ * (c + 1))
        mp = masks_p[BSL * c:BSL * (c + 1)]                       # [BSL, NM, HW]
        mQ = np.zeros((NQ, 128, HW), dtype=np.float32)
        for k in range(4):
            mQ[:, 32 * k:32 * k + NM, :] = mp.reshape(NQ, 4, NM, HW)[:, k]
        in_maps.append({
            "xlo": xlo[qsl], "xhi": xhi[qsl], "mQ": mQ,
            "w1": w1b, "w2": w2b, "b2": b2b, "gb": gb, "i128": i128,
        })

    res = bass_utils.run_bass_kernel_spmd(nc, in_maps, core_ids=list(range(N_CORES)))
    LAST_RESULTS = res

    xs = np.empty((BS, NM, OUT), dtype=np.float32)
    for c in range(N_CORES):
        shard = res.results[c]["out"]                             # [T, OUT]
        for p in range(NPROJ):
            cp = int(counts[p])
            if cp == 0:
                continue
            blk = shard[32 * offs[p]:32 * offs[p] + 32 * cp].reshape(BSL, cp, OUT)
            xs[BSL * c:BSL * (c + 1), perm[offs[p]:offs[p] + cp], :] = blk
    return xs, np.asarray(mask_ids)
